# revision 1
# baseline (speedup 1.0000x reference)
"""Trainium2 Bass kernel for AgentCapabilityEstimator (dense MLP, 3 heads).

Reference computation (B=16384, OBS=512, H=1024, N=9):
    g  = relu(relu(obs @ W1 + b1) @ W2 + b2)                    [B, H]
    cov  = sigmoid(relu(g @ Wc1 + bc1) @ Wc2 + bc2)             [B, 1]
    trk  = sigmoid(relu(g @ Wt1 + bt1) @ Wt2 + bt2)             [B, 1]
    coop = sigmoid(relu([g,g] @ Wk1 + bk1) @ Wk2 + bk2)         [B, 1]
    outputs broadcast to [B, 9] each.

Strategy: pure data parallelism over 8 cores (2048 rows each). Activations
kept feature-major ([feature_chunk=128 partitions, batch free dim]) so every
layer is a chain of 128x128 stationary x [128, 512] moving f32r matmuls with
natural-layout weights. Host-side prep folds Wk1 ([g,g] @ Wk1 == g @ (Wk1_hi
+ Wk1_lo)), concatenates the three head hidden layers into one [1024, 2048]
GEMM, and packs the three scalar head outputs into one [2048, 3] block-sparse
final matmul.
"""

import numpy as np

import concourse.bass as bass
import concourse.mybir as mybir
import concourse.tile as tile
from concourse import bacc
from concourse.bass_utils import run_bass_kernel_spmd
from concourse.masks import make_identity

B, OBS, H, N = 16384, 512, 1024, 9
NCORES = 8
BC = B // NCORES          # 2048 batch rows per core
NTILE = 512               # batch rows per compute tile
NT = BC // NTILE          # 4 tiles per core
P = 128
KO = OBS // P             # 4 obs chunks
HO = H // P               # 8 hidden chunks
AO = 2 * H // P           # 16 chunks of the stacked head-hidden features
F32 = mybir.dt.float32
F32R = mybir.dt.float32r

RELU = mybir.ActivationFunctionType.Relu
SIGMOID = mybir.ActivationFunctionType.Sigmoid


def build_nc() -> bass.Bass:
    nc = bacc.Bacc(trn_type="TRN2", target_bir_lowering=False, debug=False)

    obsT = nc.dram_tensor("obsT", [OBS, BC], F32R, kind="ExternalInput").ap()
    W1 = nc.dram_tensor("W1", [OBS, H], F32R, kind="ExternalInput").ap()
    W2 = nc.dram_tensor("W2", [H, H], F32R, kind="ExternalInput").ap()
    Wh = nc.dram_tensor("Wh", [H, 2 * H], F32R, kind="ExternalInput").ap()
    Wfin = nc.dram_tensor("Wfin", [2 * H, 3], F32R, kind="ExternalInput").ap()
    b1 = nc.dram_tensor("b1", [H], F32, kind="ExternalInput").ap()
    b2 = nc.dram_tensor("b2", [H], F32, kind="ExternalInput").ap()
    bh = nc.dram_tensor("bh", [2 * H], F32, kind="ExternalInput").ap()
    bfin = nc.dram_tensor("bfin", [3], F32, kind="ExternalInput").ap()
    out_cov = nc.dram_tensor("cov", [BC, N], F32, kind="ExternalOutput").ap()
    out_trk = nc.dram_tensor("trk", [BC, N], F32, kind="ExternalOutput").ap()
    out_coop = nc.dram_tensor("coop", [BC, N], F32, kind="ExternalOutput").ap()

    with tile.TileContext(nc) as tc:
        _body(tc, obsT, W1, W2, Wh, Wfin, b1, b2, bh, bfin,
              out_cov, out_trk, out_coop)
    nc.compile()
    return nc


def _body(tc, obsT, W1, W2, Wh, Wfin, b1, b2, bh, bfin,
          out_cov, out_trk, out_coop):
    nc = tc.nc

    with (
        tc.tile_pool(name="weights", bufs=1) as wpool,
        tc.tile_pool(name="obs", bufs=2) as obspool,
        tc.tile_pool(name="acts", bufs=1) as actpool,
        tc.tile_pool(name="hpool", bufs=4) as hpool,
        tc.tile_pool(name="gpool", bufs=2) as gpool,
        tc.tile_pool(name="small", bufs=2) as smallpool,
        tc.tile_pool(name="psum", bufs=5, space="PSUM") as psum,
        tc.tile_pool(name="psum_fin", bufs=1, space="PSUM") as psum_f,
        tc.tile_pool(name="psum_tr", bufs=2, space="PSUM") as psum_t,
    ):
        # ---- resident weights / biases ----------------------------------
        # DMAs are split into ~0.5-1MB chunks and issued in the order compute
        # consumes them, so early-phase matmuls are not stuck behind
        # later-phase weight traffic on shared HBM bandwidth.
        obsT_r = obsT.rearrange("(c p) b -> p c b", p=P)
        w1_sb = wpool.tile([P, KO, H], F32R)
        w2_sb = wpool.tile([P, HO, H], F32R)
        wh_sb = wpool.tile([P, HO, 2 * H], F32R)
        W1_r = W1.rearrange("(c p) h -> p c h", p=P)
        W2_r = W2.rearrange("(c p) h -> p c h", p=P)
        Wh_r = Wh.rearrange("(c p) h -> p c h", p=P)

        # phase-1 needs: obs tile 0 + W1 + b1
        xs = {0: obspool.tile([P, KO, NTILE], F32R, tag="x", name="x0")}
        for k in range(KO):
            nc.sync.dma_start(out=xs[0][:, k, :], in_=obsT_r[:, k, 0:NTILE])
            nc.sync.dma_start(out=w1_sb[:, k, :], in_=W1_r[:, k, :])
        b1_sb = wpool.tile([P, HO], F32)
        nc.sync.dma_start(out=b1_sb, in_=b1.rearrange("(c p) -> p c", p=P))
        # phase-2: W2 + b2
        for k in range(HO):
            nc.sync.dma_start(out=w2_sb[:, k, :], in_=W2_r[:, k, :])
        b2_sb = wpool.tile([P, HO], F32)
        nc.sync.dma_start(out=b2_sb, in_=b2.rearrange("(c p) -> p c", p=P))
        # phase-3: Wh (1MB per k-chunk, contiguous 8KB per partition)
        for k in range(HO):
            nc.sync.dma_start(out=wh_sb[:, k, :], in_=Wh_r[:, k, :])
        bh_sb = wpool.tile([P, AO], F32)
        nc.sync.dma_start(out=bh_sb, in_=bh.rearrange("(c p) -> p c", p=P))
        # phase-4: finals
        wfin_sb = wpool.tile([P, AO, 3], F32R)
        nc.sync.dma_start(out=wfin_sb, in_=Wfin.rearrange("(c p) m -> p c m", p=P))
        bfin_sb = wpool.tile([3, 1], F32)
        nc.sync.dma_start(out=bfin_sb, in_=bfin.rearrange("(m o) -> m o", o=1))
        ident = wpool.tile([P, P], F32)
        make_identity(nc, ident)

        gs = {}

        def trunk(t):
            bs = t * NTILE
            if t not in xs:
                xs[t] = obspool.tile([P, KO, NTILE], F32R, tag="x",
                                     name=f"x{t}")
                for k in range(KO):
                    nc.sync.dma_start(out=xs[t][:, k, :],
                                      in_=obsT_r[:, k, bs:bs + NTILE])
            x = xs[t]
            # layer 1: g1 = relu(W1.T @ x + b1)
            g1 = actpool.tile([P, HO, NTILE], F32R, tag="g1")
            for m in range(HO):
                ps = psum.tile([P, NTILE], F32, tag="mm")
                for k in range(KO):
                    nc.tensor.matmul(
                        ps, w1_sb[:, k, m * P:(m + 1) * P], x[:, k, :],
                        start=(k == 0), stop=(k == KO - 1))
                nc.scalar.activation(g1[:, m, :], ps, RELU,
                                     bias=b1_sb[:, m:m + 1])
            # layer 2: g = relu(W2.T @ g1 + b2)
            g = gpool.tile([P, HO, NTILE], F32R, tag="g")
            for m in range(HO):
                ps = psum.tile([P, NTILE], F32, tag="mm")
                for k in range(HO):
                    nc.tensor.matmul(
                        ps, w2_sb[:, k, m * P:(m + 1) * P], g1[:, k, :],
                        start=(k == 0), stop=(k == HO - 1))
                nc.scalar.activation(g[:, m, :], ps, RELU,
                                     bias=b2_sb[:, m:m + 1])
            gs[t] = g

        def heads(t):
            bs = t * NTILE
            g = gs.pop(t)
            # head hiddens h = relu(Wh.T @ g + bh), streamed per m-chunk;
            # the final-layer matmul consumes each chunk immediately so h
            # never needs to be fully resident.
            ps3 = psum_f.tile([3, NTILE], F32, tag="fin")
            for m in range(AO):
                ps = psum.tile([P, NTILE], F32, tag="mm")
                for k in range(HO):
                    nc.tensor.matmul(
                        ps, wh_sb[:, k, m * P:(m + 1) * P], g[:, k, :],
                        start=(k == 0), stop=(k == HO - 1))
                hm = hpool.tile([P, NTILE], F32R, tag="h", name=f"h{t}_{m}")
                nc.scalar.activation(hm, ps, RELU, bias=bh_sb[:, m:m + 1])
                nc.tensor.matmul(ps3, wfin_sb[:, m, :], hm,
                                 start=(m == 0), stop=(m == AO - 1))
            sig = smallpool.tile([3, NTILE], F32, tag="sig")
            nc.scalar.activation(sig, ps3, SIGMOID, bias=bfin_sb[0:3, 0:1])
            # transpose to batch-major, broadcast to 9, store
            for c in range(NTILE // P):
                pst = psum_t.tile([P, 3], F32, tag="tr")
                nc.tensor.transpose(pst, sig[:, c * P:(c + 1) * P],
                                    ident[0:3, 0:3])
                o27 = smallpool.tile([P, 3, N], F32, tag="o27")
                for i in range(3):
                    nc.vector.tensor_copy(
                        out=o27[:, i, :],
                        in_=pst[:, i:i + 1].broadcast_to([P, N]))
                rows = slice(bs + c * P, bs + (c + 1) * P)
                nc.sync.dma_start(out=out_cov[rows, :], in_=o27[:, 0, :])
                nc.sync.dma_start(out=out_trk[rows, :], in_=o27[:, 1, :])
                nc.sync.dma_start(out=out_coop[rows, :], in_=o27[:, 2, :])

        # depth-2 software pipeline: head phases (which need the large Wh)
        # trail trunk phases by two tiles so Wh's DMA hides behind compute.
        trunk(0)
        trunk(1)
        heads(0)
        trunk(2)
        heads(1)
        trunk(3)
        heads(2)
        heads(3)


_NC_CACHE = None


def _get_nc() -> bass.Bass:
    global _NC_CACHE
    if _NC_CACHE is None:
        _NC_CACHE = build_nc()
    return _NC_CACHE


def prep_inputs(obs, W1, b1, W2, b2, Wc1, bc1, Wc2, bc2,
                Wt1, bt1, Wt2, bt2, Wk1, bk1, Wk2, bk2, **_unused):
    """Host-side prep: fold/concat weights, transpose obs, build shards."""
    f = np.float32
    obsT = np.ascontiguousarray(np.asarray(obs, f).T)          # [OBS, B]
    Wk1f = np.asarray(Wk1[:H], f) + np.asarray(Wk1[H:], f)     # [H, H]
    Wh = np.ascontiguousarray(
        np.concatenate([np.asarray(Wc1, f), np.asarray(Wt1, f), Wk1f],
                       axis=1))                                # [H, 2H]
    Wfin = np.zeros((2 * H, 3), f)
    Wfin[0:H // 2, 0] = np.asarray(Wc2, f)[:, 0]
    Wfin[H // 2:H, 1] = np.asarray(Wt2, f)[:, 0]
    Wfin[H:2 * H, 2] = np.asarray(Wk2, f)[:, 0]
    bh = np.concatenate([np.asarray(bc1, f), np.asarray(bt1, f),
                         np.asarray(bk1, f)])                  # [2H]
    bfin = np.array([np.asarray(bc2, f)[0], np.asarray(bt2, f)[0],
                     np.asarray(bk2, f)[0]], f)

    shared = dict(
        W1=np.ascontiguousarray(np.asarray(W1, f)),
        W2=np.ascontiguousarray(np.asarray(W2, f)),
        Wh=Wh, Wfin=Wfin,
        b1=np.ascontiguousarray(np.asarray(b1, f)),
        b2=np.ascontiguousarray(np.asarray(b2, f)),
        bh=np.ascontiguousarray(bh), bfin=bfin,
    )
    in_maps = []
    for c in range(NCORES):
        m = dict(shared)
        m["obsT"] = np.ascontiguousarray(obsT[:, c * BC:(c + 1) * BC])
        in_maps.append(m)
    return in_maps


def kernel(**inputs):
    nc = _get_nc()
    in_maps = prep_inputs(**inputs)
    res = run_bass_kernel_spmd(nc, in_maps, list(range(NCORES))).results
    cov = np.concatenate([res[c]["cov"] for c in range(NCORES)], axis=0)
    trk = np.concatenate([res[c]["trk"] for c in range(NCORES)], axis=0)
    coop = np.concatenate([res[c]["coop"] for c in range(NCORES)], axis=0)
    return (cov, trk, coop)



# revision 7
# speedup vs baseline: 1.9564x; 1.9564x over previous
"""Trainium2 Bass kernel for AgentCapabilityEstimator (dense MLP, 3 heads).

Reference computation (B=16384, OBS=512, H=1024, N=9):
    g  = relu(relu(obs @ W1 + b1) @ W2 + b2)                    [B, H]
    cov  = sigmoid(relu(g @ Wc1 + bc1) @ Wc2 + bc2)             [B, 1]
    trk  = sigmoid(relu(g @ Wt1 + bt1) @ Wt2 + bt2)             [B, 1]
    coop = sigmoid(relu([g,g] @ Wk1 + bk1) @ Wk2 + bk2)         [B, 1]
    outputs broadcast to [B, 9] each.

Strategy: pure data parallelism over 8 cores (2048 rows each), with all four
GEMM stages in fp8(e4m3) DoubleRow mode (2 fp8 MACs per PE per cycle, 256-deep
contraction per matmul).  Weights are scaled by 64 host-side so they sit in
e4m3's normal range; each layer's activation applies a compile-time descale
and a power-of-two activation quantisation scale before casting back to fp8.
The final sigmoid applies the exact inverse scale, so only quantisation noise
(<1e-2 on this model, threshold 2e-2) remains.  Relu/quant work is split
across the scalar and vector engines so neither blocks the tensor engine.
Host-side prep folds Wk1 ([g,g] @ Wk1 == g @ (Wk1_hi + Wk1_lo)), concatenates
the three head hidden layers into one [1024, 2048] GEMM, and packs the three
scalar head outputs into one [2048, 3] block-sparse final matmul.
"""

import numpy as np
import ml_dtypes

import concourse.bass as bass
import concourse.mybir as mybir
import concourse.tile as tile
from concourse import bacc
from concourse.bass_utils import run_bass_kernel_spmd
from concourse.masks import make_identity

B, OBS, H, N = 16384, 512, 1024, 9
NCORES = 8
BC = B // NCORES          # 2048 batch rows per core
NTILE = 512               # batch rows per compute tile
HALF = 256                # batch rows per DoubleRow matmul (moving 2x256=512)
NT = BC // NTILE          # 4 tiles per core
P = 128
KO = OBS // P             # 4 obs chunks
HO = H // P               # 8 hidden chunks
AO = 2 * H // P           # 16 chunks of the stacked head-hidden features
FINW = 32                 # final-matmul stationary width (3 live + zero pad;
                          # DoubleRow Ldweights ISA requires a full 32-wide tile)
F32 = mybir.dt.float32
F8 = mybir.dt.float8e4
NP_F8 = ml_dtypes.float8_e4m3

# fp8 scaling: weights x64 (e4m3 normal range), activations quantised at
# power-of-two scales S; descale constants are exact in fp32.
WS = 64.0
S1, S2, S3 = 4.0, 8.0, 16.0
DS1 = S1 / WS             # psum1 * DS1 = S1 * z1
DS2 = S2 / (WS * S1)
DS3 = S3 / (WS * S2)
DSF = 1.0 / (WS * S3)     # final psum * DSF = true logit

RELU = mybir.ActivationFunctionType.Relu
SIGMOID = mybir.ActivationFunctionType.Sigmoid
DR = mybir.MatmulPerfMode.DoubleRow
MULT = mybir.AluOpType.mult
MAX = mybir.AluOpType.max


def build_nc(zero_bias: bool) -> bass.Bass:
    nc = bacc.Bacc(trn_type="TRN2", target_bir_lowering=False, debug=False)

    obsT = nc.dram_tensor("obsT", [OBS, BC], F8, kind="ExternalInput").ap()
    W1 = nc.dram_tensor("W1", [OBS, H], F8, kind="ExternalInput").ap()
    W2 = nc.dram_tensor("W2", [H, H], F8, kind="ExternalInput").ap()
    Wh = nc.dram_tensor("Wh", [H, 2 * H], F8, kind="ExternalInput").ap()
    Wfin = nc.dram_tensor("Wfin", [2 * H, FINW], F8, kind="ExternalInput").ap()
    b1 = nc.dram_tensor("b1", [H], F32, kind="ExternalInput").ap()
    b2 = nc.dram_tensor("b2", [H], F32, kind="ExternalInput").ap()
    bh = nc.dram_tensor("bh", [2 * H], F32, kind="ExternalInput").ap()
    bfin = nc.dram_tensor("bfin", [3], F32, kind="ExternalInput").ap()
    out_cov = nc.dram_tensor("cov", [BC, N], F32, kind="ExternalOutput").ap()
    out_trk = nc.dram_tensor("trk", [BC, N], F32, kind="ExternalOutput").ap()
    out_coop = nc.dram_tensor("coop", [BC, N], F32, kind="ExternalOutput").ap()

    with tile.TileContext(nc) as tc:
        _body(tc, zero_bias, obsT, W1, W2, Wh, Wfin, b1, b2, bh, bfin,
              out_cov, out_trk, out_coop)
    nc.compile()
    return nc


def _body(tc, zero_bias, obsT, W1, W2, Wh, Wfin, b1, b2, bh, bfin,
          out_cov, out_trk, out_coop):
    nc = tc.nc

    with (
        tc.tile_pool(name="weights", bufs=1) as wpool,
        tc.tile_pool(name="obs", bufs=2) as obspool,
        tc.tile_pool(name="acts", bufs=1) as actpool,
        tc.tile_pool(name="hpool", bufs=3) as hpool,
        tc.tile_pool(name="gpool", bufs=2) as gpool,
        tc.tile_pool(name="small", bufs=2) as smallpool,
        tc.tile_pool(name="psum", bufs=4, space="PSUM") as psum,
        tc.tile_pool(name="psum_fin", bufs=2, space="PSUM") as psum_f,
        tc.tile_pool(name="psum_tr", bufs=2, space="PSUM") as psum_t,
    ):
        # ---- resident weights / biases ----------------------------------
        # DMAs issued in the order compute consumes them so early-phase
        # matmuls are not stuck behind later-phase weight traffic.
        obsT_r = obsT.rearrange("(c p) b -> p c b", p=P)
        w1_sb = wpool.tile([P, KO, H], F8)
        w2_sb = wpool.tile([P, HO, H], F8)
        wh_sb = wpool.tile([P, HO, 2 * H], F8)
        W1_r = W1.rearrange("(c p) h -> p c h", p=P)
        W2_r = W2.rearrange("(c p) h -> p c h", p=P)
        Wh_r = Wh.rearrange("(c p) h -> p c h", p=P)

        # phase-1 needs: obs tile 0 + W1 + b1
        xs = {0: obspool.tile([P, KO, NTILE], F8, tag="x", name="x0")}
        for k in range(KO):
            nc.sync.dma_start(out=xs[0][:, k, :], in_=obsT_r[:, k, 0:NTILE])
        nc.sync.dma_start(out=w1_sb, in_=W1_r)
        b1_sb = wpool.tile([P, HO], F32)
        nc.sync.dma_start(out=b1_sb, in_=b1.rearrange("(c p) -> p c", p=P))
        # phase-2: W2 + b2
        for k in range(HO):
            nc.sync.dma_start(out=w2_sb[:, k, :], in_=W2_r[:, k, :])
        b2_sb = wpool.tile([P, HO], F32)
        nc.sync.dma_start(out=b2_sb, in_=b2.rearrange("(c p) -> p c", p=P))
        # phase-3: Wh
        for k in range(HO):
            nc.sync.dma_start(out=wh_sb[:, k, :], in_=Wh_r[:, k, :])
        bh_sb = wpool.tile([P, AO], F32)
        nc.sync.dma_start(out=bh_sb, in_=bh.rearrange("(c p) -> p c", p=P))
        # phase-4: finals
        wfin_sb = wpool.tile([P, AO, FINW], F8)
        nc.sync.dma_start(out=wfin_sb, in_=Wfin.rearrange("(c p) m -> p c m", p=P))
        bfin_sb = wpool.tile([3, 1], F32)
        nc.sync.dma_start(out=bfin_sb, in_=bfin.rearrange("(m o) -> m o", o=1))
        ident = wpool.tile([P, P], F32)
        make_identity(nc, ident)

        def act_relu(out, ps, bias_sb, ds, use_dve):
            """out_fp8 = S*relu(z+b): scalar path relu(ps*ds + S*b),
            DVE path (zero bias only) max(ps*ds, 0)."""
            if use_dve and zero_bias:
                nc.vector.tensor_scalar(out, ps, ds, 0.0, MULT, MAX)
            else:
                nc.scalar.activation(out, ps, RELU, bias=bias_sb, scale=ds)

        gs = {}

        def trunk(t):
            bs = t * NTILE
            if t not in xs:
                xs[t] = obspool.tile([P, KO, NTILE], F8, tag="x",
                                     name=f"x{t}")
                for k in range(KO):
                    nc.sync.dma_start(out=xs[t][:, k, :],
                                      in_=obsT_r[:, k, bs:bs + NTILE])
            x = xs[t]
            # layer 1: g1 = S1*relu(W1.T @ x + b1)
            g1 = actpool.tile([P, HO, NTILE], F8, tag="g1")
            for m in range(HO):
                ps = psum.tile([P, NTILE], F32, tag="mm")
                for h in range(2):
                    cols = slice(h * HALF, (h + 1) * HALF)
                    for k in range(KO // 2):
                        nc.tensor.matmul(
                            ps[:, cols],
                            w1_sb[:, 2 * k:2 * k + 2, m * P:(m + 1) * P],
                            x[:, 2 * k:2 * k + 2, cols],
                            start=(k == 0), stop=(k == KO // 2 - 1),
                            perf_mode=DR)
                act_relu(g1[:, m, :], ps, b1_sb[:, m:m + 1], DS1, m % 2 == 1)
            # layer 2: g = S2*relu(W2.T @ g1 + b2)
            g = gpool.tile([P, HO, NTILE], F8, tag="g")
            for m in range(HO):
                ps = psum.tile([P, NTILE], F32, tag="mm")
                for h in range(2):
                    cols = slice(h * HALF, (h + 1) * HALF)
                    for k in range(HO // 2):
                        nc.tensor.matmul(
                            ps[:, cols],
                            w2_sb[:, 2 * k:2 * k + 2, m * P:(m + 1) * P],
                            g1[:, 2 * k:2 * k + 2, cols],
                            start=(k == 0), stop=(k == HO // 2 - 1),
                            perf_mode=DR)
                act_relu(g[:, m, :], ps, b2_sb[:, m:m + 1], DS2, m % 2 == 1)
            gs[t] = g

        def heads(t):
            bs = t * NTILE
            g = gs.pop(t)
            # head hiddens h = S3*relu(Wh.T @ g + bh), produced in chunk
            # pairs; each pair feeds one DoubleRow final matmul so h never
            # needs to be fully resident.  The two batch halves' fin
            # accumulation groups are interleaved in time, so each needs its
            # own full PSUM bank: a `start` marks the whole 2KB zero region
            # pending and would wipe the other half's partial accumulation.
            ps3 = [psum_f.tile([FINW, NTILE], F32, tag="fin", name=f"fin{t}_{h}")
                   for h in range(2)]
            h2 = None
            for m in range(AO):
                if m % 2 == 0:
                    h2 = hpool.tile([P, 2, NTILE], F8, tag="h",
                                    name=f"h{t}_{m // 2}")
                ps = psum.tile([P, NTILE], F32, tag="mm")
                for h in range(2):
                    cols = slice(h * HALF, (h + 1) * HALF)
                    for k in range(HO // 2):
                        nc.tensor.matmul(
                            ps[:, cols],
                            wh_sb[:, 2 * k:2 * k + 2, m * P:(m + 1) * P],
                            g[:, 2 * k:2 * k + 2, cols],
                            start=(k == 0), stop=(k == HO // 2 - 1),
                            perf_mode=DR)
                act_relu(h2[:, m % 2, :], ps, bh_sb[:, m:m + 1], DS3,
                         m % 2 == 1)
                if m % 2 == 1:
                    for h in range(2):
                        cols = slice(h * HALF, (h + 1) * HALF)
                        nc.tensor.matmul(
                            ps3[h][:, cols],
                            wfin_sb[:, m - 1:m + 1, :],
                            h2[:, :, cols],
                            start=(m == 1), stop=(m == AO - 1),
                            perf_mode=DR)
            sig = smallpool.tile([3, NTILE], F32, tag="sig")
            for h in range(2):
                cols = slice(h * HALF, (h + 1) * HALF)
                nc.scalar.activation(sig[:, cols], ps3[h][0:3, cols],
                                     SIGMOID, bias=bfin_sb[0:3, 0:1],
                                     scale=DSF)
            # transpose to batch-major, broadcast to 9, store
            for c in range(NTILE // P):
                pst = psum_t.tile([P, 3], F32, tag="tr")
                nc.tensor.transpose(pst, sig[:, c * P:(c + 1) * P],
                                    ident[0:3, 0:3])
                o27 = smallpool.tile([P, 3, N], F32, tag="o27")
                for i in range(3):
                    nc.vector.tensor_copy(
                        out=o27[:, i, :],
                        in_=pst[:, i:i + 1].broadcast_to([P, N]))
                rows = slice(bs + c * P, bs + (c + 1) * P)
                nc.sync.dma_start(out=out_cov[rows, :], in_=o27[:, 0, :])
                nc.sync.dma_start(out=out_trk[rows, :], in_=o27[:, 1, :])
                nc.sync.dma_start(out=out_coop[rows, :], in_=o27[:, 2, :])

        # depth-2 software pipeline: head phases (which need the large Wh)
        # trail trunk phases by two tiles so Wh's DMA hides behind compute.
        trunk(0)
        trunk(1)
        heads(0)
        trunk(2)
        heads(1)
        trunk(3)
        heads(2)
        heads(3)


_NC_CACHE = {}


def _get_nc(zero_bias: bool = True) -> bass.Bass:
    if zero_bias not in _NC_CACHE:
        _NC_CACHE[zero_bias] = build_nc(zero_bias)
    return _NC_CACHE[zero_bias]


def prep_inputs(obs, W1, b1, W2, b2, Wc1, bc1, Wc2, bc2,
                Wt1, bt1, Wt2, bt2, Wk1, bk1, Wk2, bk2, **_unused):
    """Host-side prep: fold/concat weights, scale + quantise to fp8,
    transpose obs, build shards."""
    f = np.float32

    def q8(a):
        return np.ascontiguousarray(
            np.clip(np.asarray(a, f), -240.0, 240.0).astype(NP_F8))

    obsT = np.asarray(obs, f).T                                # [OBS, B]
    Wk1f = np.asarray(Wk1[:H], f) + np.asarray(Wk1[H:], f)     # [H, H]
    Wh = np.concatenate([np.asarray(Wc1, f), np.asarray(Wt1, f), Wk1f],
                        axis=1)                                # [H, 2H]
    Wfin = np.zeros((2 * H, FINW), f)
    Wfin[0:H // 2, 0] = np.asarray(Wc2, f)[:, 0]
    Wfin[H // 2:H, 1] = np.asarray(Wt2, f)[:, 0]
    Wfin[H:2 * H, 2] = np.asarray(Wk2, f)[:, 0]
    bhc = np.concatenate([np.asarray(bc1, f), np.asarray(bt1, f),
                          np.asarray(bk1, f)])                 # [2H]
    bfin = np.array([np.asarray(bc2, f)[0], np.asarray(bt2, f)[0],
                     np.asarray(bk2, f)[0]], f)

    # biases folded at each layer's activation quantisation scale
    b1q = (S1 * np.asarray(b1, f)).astype(f)
    b2q = (S2 * np.asarray(b2, f)).astype(f)
    bhq = (S3 * bhc).astype(f)
    zero_bias = not (b1q.any() or b2q.any() or bhq.any())

    obsT_q = np.clip(obsT, -240.0, 240.0).astype(NP_F8)
    shared = dict(
        W1=q8(np.asarray(W1, f) * WS),
        W2=q8(np.asarray(W2, f) * WS),
        Wh=q8(Wh * WS), Wfin=q8(Wfin * WS),
        b1=np.ascontiguousarray(b1q), b2=np.ascontiguousarray(b2q),
        bh=np.ascontiguousarray(bhq), bfin=bfin,
    )
    in_maps = []
    for c in range(NCORES):
        m = dict(shared)
        m["obsT"] = np.ascontiguousarray(obsT_q[:, c * BC:(c + 1) * BC])
        in_maps.append(m)
    return in_maps, zero_bias


def kernel(**inputs):
    in_maps, zero_bias = prep_inputs(**inputs)
    nc = _get_nc(zero_bias)
    res = run_bass_kernel_spmd(nc, in_maps, list(range(NCORES))).results
    cov = np.concatenate([res[c]["cov"] for c in range(NCORES)], axis=0)
    trk = np.concatenate([res[c]["trk"] for c in range(NCORES)], axis=0)
    coop = np.concatenate([res[c]["coop"] for c in range(NCORES)], axis=0)
    return (cov, trk, coop)


# revision 20
# speedup vs baseline: 1.9987x; 1.0216x over previous
"""Trainium2 Bass kernel for AgentCapabilityEstimator (dense MLP, 3 heads).

Reference computation (B=16384, OBS=512, H=1024, N=9):
    g  = relu(relu(obs @ W1 + b1) @ W2 + b2)                    [B, H]
    cov  = sigmoid(relu(g @ Wc1 + bc1) @ Wc2 + bc2)             [B, 1]
    trk  = sigmoid(relu(g @ Wt1 + bt1) @ Wt2 + bt2)             [B, 1]
    coop = sigmoid(relu([g,g] @ Wk1 + bk1) @ Wk2 + bk2)         [B, 1]
    outputs broadcast to [B, 9] each.

Strategy: pure data parallelism over 8 cores (2048 rows each), with all four
GEMM stages in fp8(e4m3) DoubleRow mode (2 fp8 MACs per PE per cycle, 256-deep
contraction per matmul).  Weights are scaled by 64 host-side so they sit in
e4m3's normal range; each layer's activation applies a compile-time descale
and a power-of-two activation quantisation scale before casting back to fp8.
The final sigmoid applies the exact inverse scale, so only quantisation noise
(<1e-2 on this model, threshold 2e-2) remains.  Relu/quant work is split
across the scalar and vector engines so neither blocks the tensor engine.
Host-side prep folds Wk1 ([g,g] @ Wk1 == g @ (Wk1_hi + Wk1_lo)), concatenates
the three head hidden layers into one [1024, 2048] GEMM, and packs the three
scalar head outputs into one [2048, 3] block-sparse final matmul.
"""

import numpy as np
import ml_dtypes

import concourse.bass as bass
import concourse.mybir as mybir
import concourse.tile as tile
from concourse import bacc
from concourse.bass_utils import run_bass_kernel_spmd
from concourse.masks import make_identity

B, OBS, H, N = 16384, 512, 1024, 9
NCORES = 8
BC = B // NCORES          # 2048 batch rows per core
NTILE = 512               # batch rows per compute tile
HALF = 256                # batch rows per DoubleRow matmul (moving 2x256=512)
NT = BC // NTILE          # 4 tiles per core
P = 128
KO = OBS // P             # 4 obs chunks
HO = H // P               # 8 hidden chunks
AO = 2 * H // P           # 16 chunks of the stacked head-hidden features
FINW = 32                 # final-matmul stationary width (3 live + zero pad;
                          # DoubleRow Ldweights ISA requires a full 32-wide tile)
F32 = mybir.dt.float32
F8 = mybir.dt.float8e4
NP_F8 = ml_dtypes.float8_e4m3

# fp8 scaling: weights x64 (e4m3 normal range), activations quantised at
# power-of-two scales S; descale constants are exact in fp32.
WS = 64.0
S1, S2, S3 = 4.0, 8.0, 16.0
DS1 = S1 / WS             # psum1 * DS1 = S1 * z1
DS2 = S2 / (WS * S1)
DS3 = S3 / (WS * S2)
DSF = 1.0 / (WS * S3)     # final psum * DSF = true logit

RELU = mybir.ActivationFunctionType.Relu
SIGMOID = mybir.ActivationFunctionType.Sigmoid
DR = mybir.MatmulPerfMode.DoubleRow
MULT = mybir.AluOpType.mult
MAX = mybir.AluOpType.max


def build_nc(zero_bias: bool) -> bass.Bass:
    nc = bacc.Bacc(trn_type="TRN2", target_bir_lowering=False, debug=False)

    # All tensors are pre-arranged host-side to partition-major layouts so
    # every DMA is 128 large contiguous runs (one per partition) instead of
    # thousands of sub-KB descriptors on one hardware queue.  Input DMAs are
    # split across both hardware DGE queues: obs/W1 on the SP queue (first
    # matmul's critical path), later-phase weights on the Activation queue.
    obsA = nc.dram_tensor("obsA", [P, NT * KO * NTILE], F8,
                          kind="ExternalInput").ap()
    W1 = nc.dram_tensor("W1A", [P, KO * H], F8, kind="ExternalInput").ap()
    W2 = nc.dram_tensor("W2A", [P, HO * H], F8, kind="ExternalInput").ap()
    Wh = nc.dram_tensor("WhA", [P, HO * 2 * H], F8, kind="ExternalInput").ap()
    Wfin = nc.dram_tensor("WfinA", [P, AO * FINW], F8,
                          kind="ExternalInput").ap()
    b1 = nc.dram_tensor("b1A", [P, HO], F32, kind="ExternalInput").ap()
    b2 = nc.dram_tensor("b2A", [P, HO], F32, kind="ExternalInput").ap()
    bh = nc.dram_tensor("bhA", [P, AO], F32, kind="ExternalInput").ap()
    bfin = nc.dram_tensor("bfin", [3], F32, kind="ExternalInput").ap()
    out_cov = nc.dram_tensor("cov", [BC, N], F32, kind="ExternalOutput").ap()
    out_trk = nc.dram_tensor("trk", [BC, N], F32, kind="ExternalOutput").ap()
    out_coop = nc.dram_tensor("coop", [BC, N], F32, kind="ExternalOutput").ap()

    with tile.TileContext(nc) as tc:
        _body(tc, zero_bias, obsA, W1, W2, Wh, Wfin, b1, b2, bh, bfin,
              out_cov, out_trk, out_coop)
    nc.compile()
    return nc


def _body(tc, zero_bias, obsA, W1, W2, Wh, Wfin, b1, b2, bh, bfin,
          out_cov, out_trk, out_coop):
    nc = tc.nc

    with (
        tc.tile_pool(name="weights", bufs=1) as wpool,
        tc.tile_pool(name="obs", bufs=4) as obspool,
        tc.tile_pool(name="acts", bufs=3) as actpool,
        tc.tile_pool(name="hpool", bufs=3) as hpool,
        tc.tile_pool(name="gpool", bufs=3) as gpool,
        tc.tile_pool(name="small", bufs=2) as smallpool,
        tc.tile_pool(name="psum", bufs=4, space="PSUM") as psum,
        tc.tile_pool(name="psum_fin", bufs=2, space="PSUM") as psum_f,
        tc.tile_pool(name="psum_tr", bufs=2, space="PSUM") as psum_t,
    ):
        # ---- resident weights / biases ----------------------------------
        obsA_r = obsA.rearrange("p (t k j) -> p t k j", t=NT, k=KO)
        w1_sb = wpool.tile([P, KO, H], F8)
        w2_sb = wpool.tile([P, HO, H], F8)
        wh_sb = wpool.tile([P, HO, 2 * H], F8)

        # SP queue: obs tile 0 + W1 + b1 (first matmul's critical path),
        # then the remaining obs tiles prefetched behind them.
        xs = {t: obspool.tile([P, KO, NTILE], F8, tag="x", name=f"x{t}")
              for t in range(NT)}
        nc.sync.dma_start(out=xs[0], in_=obsA_r[:, 0, :, :])
        nc.sync.dma_start(out=w1_sb, in_=W1.rearrange("p (c h) -> p c h", c=KO))
        b1_sb = wpool.tile([P, HO], F32)
        nc.sync.dma_start(out=b1_sb, in_=b1)
        for t in range(1, NT):
            nc.sync.dma_start(out=xs[t], in_=obsA_r[:, t, :, :])
        # Activation queue, in consumption order: W2, Wh, finals
        nc.scalar.dma_start(out=w2_sb, in_=W2.rearrange("p (c h) -> p c h", c=HO))
        b2_sb = wpool.tile([P, HO], F32)
        nc.scalar.dma_start(out=b2_sb, in_=b2)
        nc.scalar.dma_start(out=wh_sb, in_=Wh.rearrange("p (c h) -> p c h", c=HO))
        bh_sb = wpool.tile([P, AO], F32)
        nc.scalar.dma_start(out=bh_sb, in_=bh)
        wfin_sb = wpool.tile([P, AO, FINW], F8)
        nc.scalar.dma_start(out=wfin_sb,
                            in_=Wfin.rearrange("p (c m) -> p c m", c=AO))
        bfin_sb = wpool.tile([3, 1], F32)
        nc.scalar.dma_start(out=bfin_sb, in_=bfin.rearrange("(m o) -> m o", o=1))
        ident = wpool.tile([P, P], F32)
        make_identity(nc, ident)

        def act_relu(out, ps, bias_sb, ds, use_dve):
            """out_fp8 = S*relu(z+b): scalar path relu(ps*ds + S*b),
            DVE path (zero bias only) max(ps*ds, 0)."""
            if use_dve and zero_bias:
                nc.vector.tensor_scalar(out, ps, ds, 0.0, MULT, MAX)
            else:
                nc.scalar.activation(out, ps, RELU, bias=bias_sb, scale=ds)

        g1s = {}
        gs = {}

        def layer(x, w_sb, b_sb, ds, out_tag, kchunks):
            """one fused GEMM layer: out = S*relu(w.T @ x + b) in fp8"""
            out = (actpool if out_tag == "g1" else gpool).tile(
                [P, HO, NTILE], F8, tag=out_tag)
            for m in range(HO):
                ps = psum.tile([P, NTILE], F32, tag="mm")
                for h in range(2):
                    cols = slice(h * HALF, (h + 1) * HALF)
                    for k in range(kchunks // 2):
                        nc.tensor.matmul(
                            ps[:, cols],
                            w_sb[:, 2 * k:2 * k + 2, m * P:(m + 1) * P],
                            x[:, 2 * k:2 * k + 2, cols],
                            start=(k == 0), stop=(k == kchunks // 2 - 1),
                            perf_mode=DR)
                act_relu(out[:, m, :], ps, b_sb[:, m:m + 1], ds, m % 2 == 1)
            return out

        def l1(t):
            g1s[t] = layer(xs.pop(t), w1_sb, b1_sb, DS1, "g1", KO)

        def l2(t):
            gs[t] = layer(g1s.pop(t), w2_sb, b2_sb, DS2, "g", HO)

        def heads(t):
            bs = t * NTILE
            g = gs.pop(t)
            # head hiddens h = S3*relu(Wh.T @ g + bh), produced in chunk
            # pairs; each pair feeds one DoubleRow final matmul, emitted one
            # pair LATE so the pair's relu/quant (on scalar/DVE) hides behind
            # the next pair's 8 matmuls instead of stalling the PE.  The two
            # batch halves' fin accumulation groups are interleaved in time,
            # so each needs its own full PSUM bank: a `start` marks the whole
            # 2KB zero region pending and would wipe the other half's
            # partial accumulation.
            ps3 = [psum_f.tile([FINW, NTILE], F32, tag="fin", name=f"fin{t}_{h}")
                   for h in range(2)]
            h2s = {}

            def fin(j):
                for h in range(2):
                    cols = slice(h * HALF, (h + 1) * HALF)
                    nc.tensor.matmul(
                        ps3[h][:, cols],
                        wfin_sb[:, 2 * j:2 * j + 2, :],
                        h2s[j][:, :, cols],
                        start=(j == 0), stop=(j == AO // 2 - 1),
                        perf_mode=DR)

            for m in range(AO):
                j = m // 2
                if m % 2 == 0:
                    h2s[j] = hpool.tile([P, 2, NTILE], F8, tag="h",
                                        name=f"h{t}_{j}")
                ps = psum.tile([P, NTILE], F32, tag="mm")
                for h in range(2):
                    cols = slice(h * HALF, (h + 1) * HALF)
                    for k in range(HO // 2):
                        nc.tensor.matmul(
                            ps[:, cols],
                            wh_sb[:, 2 * k:2 * k + 2, m * P:(m + 1) * P],
                            g[:, 2 * k:2 * k + 2, cols],
                            start=(k == 0), stop=(k == HO // 2 - 1),
                            perf_mode=DR)
                act_relu(h2s[j][:, m % 2, :], ps, bh_sb[:, m:m + 1], DS3,
                         m % 2 == 1)
                if m % 2 == 1 and j >= 1:
                    fin(j - 1)
                    h2s.pop(j - 1)
            fin(AO // 2 - 1)
            sig = smallpool.tile([3, NTILE], F32, tag="sig")
            for h in range(2):
                cols = slice(h * HALF, (h + 1) * HALF)
                nc.scalar.activation(sig[:, cols], ps3[h][0:3, cols],
                                     SIGMOID, bias=bfin_sb[0:3, 0:1],
                                     scale=DSF)
            # transpose to batch-major, broadcast to 9, store
            for c in range(NTILE // P):
                pst = psum_t.tile([P, 3], F32, tag="tr")
                nc.tensor.transpose(pst, sig[:, c * P:(c + 1) * P],
                                    ident[0:3, 0:3])
                o27 = smallpool.tile([P, 3, N], F32, tag="o27")
                for i in range(3):
                    nc.vector.tensor_copy(
                        out=o27[:, i, :],
                        in_=pst[:, i:i + 1].broadcast_to([P, N]))
                rows = slice(bs + c * P, bs + (c + 1) * P)
                nc.sync.dma_start(out=out_cov[rows, :], in_=o27[:, 0, :])
                nc.sync.dma_start(out=out_trk[rows, :], in_=o27[:, 1, :])
                nc.sync.dma_start(out=out_coop[rows, :], in_=o27[:, 2, :])

        # Software-pipelined phase order: every layer phase is separated from
        # its producer phase by at least one unrelated phase, so the
        # producer's last relu/quant (scalar/DVE) lands well before the
        # consumer's PE matmuls need it — no layer-boundary PE stalls.
        l1(0)
        l1(1)
        l2(0)
        l1(2)
        l2(1)
        heads(0)
        l1(3)
        l2(2)
        heads(1)
        l2(3)
        heads(2)
        heads(3)


_NC_CACHE = {}


def _get_nc(zero_bias: bool = True) -> bass.Bass:
    if zero_bias not in _NC_CACHE:
        _NC_CACHE[zero_bias] = build_nc(zero_bias)
    return _NC_CACHE[zero_bias]


def prep_inputs(obs, W1, b1, W2, b2, Wc1, bc1, Wc2, bc2,
                Wt1, bt1, Wt2, bt2, Wk1, bk1, Wk2, bk2, **_unused):
    """Host-side prep: fold/concat weights, scale + quantise to fp8,
    transpose obs, build shards."""
    f = np.float32

    def q8(a):
        return np.ascontiguousarray(
            np.clip(np.asarray(a, f), -240.0, 240.0).astype(NP_F8))

    Wk1f = np.asarray(Wk1[:H], f) + np.asarray(Wk1[H:], f)     # [H, H]
    Wh = np.concatenate([np.asarray(Wc1, f), np.asarray(Wt1, f), Wk1f],
                        axis=1)                                # [H, 2H]
    Wfin = np.zeros((2 * H, FINW), f)
    Wfin[0:H // 2, 0] = np.asarray(Wc2, f)[:, 0]
    Wfin[H // 2:H, 1] = np.asarray(Wt2, f)[:, 0]
    Wfin[H:2 * H, 2] = np.asarray(Wk2, f)[:, 0]
    bhc = np.concatenate([np.asarray(bc1, f), np.asarray(bt1, f),
                          np.asarray(bk1, f)])                 # [2H]
    bfin = np.array([np.asarray(bc2, f)[0], np.asarray(bt2, f)[0],
                     np.asarray(bk2, f)[0]], f)

    # biases folded at each layer's activation quantisation scale,
    # pre-arranged to [partition, chunk]
    def barr(bq, chunks):
        return np.ascontiguousarray(
            bq.reshape(chunks, P).T.astype(f))           # [P, chunks]

    b1q = (S1 * np.asarray(b1, f)).astype(f)
    b2q = (S2 * np.asarray(b2, f)).astype(f)
    bhq = (S3 * bhc).astype(f)
    zero_bias = not (b1q.any() or b2q.any() or bhq.any())

    # weights pre-arranged to [partition, k_chunk * out]: A[p, c*out+o]
    # = Wq[c*128+p, o], so each partition's SBUF row is one contiguous run
    def warr(wq, chunks, out):
        return np.ascontiguousarray(
            wq.reshape(chunks, P, out).transpose(1, 0, 2).reshape(P, -1))

    shared = dict(
        W1A=warr(q8(np.asarray(W1, f) * WS), KO, H),
        W2A=warr(q8(np.asarray(W2, f) * WS), HO, H),
        WhA=warr(q8(Wh * WS), HO, 2 * H),
        WfinA=warr(q8(Wfin * WS), AO, FINW),
        b1A=barr(b1q, HO), b2A=barr(b2q, HO), bhA=barr(bhq, AO),
        bfin=bfin,
    )
    # obs pre-arranged per core to [p, t*KO*NTILE + k*NTILE + j]
    # = obs_q[core*BC + t*NTILE + j, k*128+p]
    obs_q = np.clip(np.asarray(obs, f), -240.0, 240.0).astype(NP_F8)
    in_maps = []
    for c in range(NCORES):
        m = dict(shared)
        blk = obs_q[c * BC:(c + 1) * BC]                 # [BC, OBS]
        m["obsA"] = np.ascontiguousarray(
            blk.reshape(NT, NTILE, KO, P).transpose(3, 0, 2, 1).reshape(P, -1))
        in_maps.append(m)
    return in_maps, zero_bias


def kernel(**inputs):
    in_maps, zero_bias = prep_inputs(**inputs)
    nc = _get_nc(zero_bias)
    res = run_bass_kernel_spmd(nc, in_maps, list(range(NCORES))).results
    cov = np.concatenate([res[c]["cov"] for c in range(NCORES)], axis=0)
    trk = np.concatenate([res[c]["trk"] for c in range(NCORES)], axis=0)
    coop = np.concatenate([res[c]["coop"] for c in range(NCORES)], axis=0)
    return (cov, trk, coop)


# revision 27
# speedup vs baseline: 2.0342x; 1.0178x over previous
"""Trainium2 Bass kernel for AgentCapabilityEstimator (dense MLP, 3 heads).

Reference computation (B=16384, OBS=512, H=1024, N=9):
    g  = relu(relu(obs @ W1 + b1) @ W2 + b2)                    [B, H]
    cov  = sigmoid(relu(g @ Wc1 + bc1) @ Wc2 + bc2)             [B, 1]
    trk  = sigmoid(relu(g @ Wt1 + bt1) @ Wt2 + bt2)             [B, 1]
    coop = sigmoid(relu([g,g] @ Wk1 + bk1) @ Wk2 + bk2)         [B, 1]
    outputs broadcast to [B, 9] each.

Strategy: pure data parallelism over 8 cores (2048 rows each), with all four
GEMM stages in fp8(e4m3) DoubleRow mode (2 fp8 MACs per PE per cycle, 256-deep
contraction per matmul).  Weights are scaled by 64 host-side so they sit in
e4m3's normal range; each layer's activation applies a compile-time descale
and a power-of-two activation quantisation scale before casting back to fp8.
The final sigmoid applies the exact inverse scale, so only quantisation noise
(<1e-2 on this model, threshold 2e-2) remains.  Relu/quant work is split
across the scalar and vector engines so neither blocks the tensor engine.
Host-side prep folds Wk1 ([g,g] @ Wk1 == g @ (Wk1_hi + Wk1_lo)), concatenates
the three head hidden layers into one [1024, 2048] GEMM, and packs the three
scalar head outputs into one [2048, 3] block-sparse final matmul.
"""

import numpy as np
import ml_dtypes

import concourse.bass as bass
import concourse.mybir as mybir
import concourse.tile as tile
from concourse import bacc
from concourse.bass_utils import run_bass_kernel_spmd
from concourse.masks import make_identity

B, OBS, H, N = 16384, 512, 1024, 9
NCORES = 8
BC = B // NCORES          # 2048 batch rows per core
NTILE = 512               # batch rows per compute tile
HALF = 256                # batch rows per DoubleRow matmul (moving 2x256=512)
NT = BC // NTILE          # 4 tiles per core
P = 128
KO = OBS // P             # 4 obs chunks
HO = H // P               # 8 hidden chunks
AO = 2 * H // P           # 16 chunks of the stacked head-hidden features
FINW = 32                 # final-matmul stationary width (3 live + zero pad;
                          # DoubleRow Ldweights ISA requires a full 32-wide tile)
F32 = mybir.dt.float32
F8 = mybir.dt.float8e4
NP_F8 = ml_dtypes.float8_e4m3

# fp8 scaling: weights x64 (e4m3 normal range), activations quantised at
# power-of-two scales S; descale constants are exact in fp32.
WS = 64.0
S1, S2, S3 = 4.0, 8.0, 16.0
DS1 = S1 / WS             # psum1 * DS1 = S1 * z1
DS2 = S2 / (WS * S1)
DS3 = S3 / (WS * S2)
DSF = 1.0 / (WS * S3)     # final psum * DSF = true logit

RELU = mybir.ActivationFunctionType.Relu
SIGMOID = mybir.ActivationFunctionType.Sigmoid
DR = mybir.MatmulPerfMode.DoubleRow
MULT = mybir.AluOpType.mult
MAX = mybir.AluOpType.max


def build_nc(zero_bias: bool) -> bass.Bass:
    nc = bacc.Bacc(trn_type="TRN2", target_bir_lowering=False, debug=False)

    # All tensors are pre-arranged host-side to partition-major layouts so
    # every DMA is 128 large contiguous runs (one per partition) instead of
    # thousands of sub-KB descriptors on one hardware queue.  Input DMAs are
    # split across both hardware DGE queues: obs/W1 on the SP queue (first
    # matmul's critical path), later-phase weights on the Activation queue.
    obsA = nc.dram_tensor("obsA", [P, NT * KO * NTILE], F8,
                          kind="ExternalInput").ap()
    W1 = nc.dram_tensor("W1A", [P, KO * H], F8, kind="ExternalInput").ap()
    W2 = nc.dram_tensor("W2A", [P, HO * H], F8, kind="ExternalInput").ap()
    Wh = nc.dram_tensor("WhA", [P, HO * 2 * H], F8, kind="ExternalInput").ap()
    Wfin = nc.dram_tensor("WfinA", [P, AO * FINW], F8,
                          kind="ExternalInput").ap()
    b1 = nc.dram_tensor("b1A", [P, HO], F32, kind="ExternalInput").ap()
    b2 = nc.dram_tensor("b2A", [P, HO], F32, kind="ExternalInput").ap()
    bh = nc.dram_tensor("bhA", [P, AO], F32, kind="ExternalInput").ap()
    bfin = nc.dram_tensor("bfin", [3], F32, kind="ExternalInput").ap()
    out_cov = nc.dram_tensor("cov", [BC, N], F32, kind="ExternalOutput").ap()
    out_trk = nc.dram_tensor("trk", [BC, N], F32, kind="ExternalOutput").ap()
    out_coop = nc.dram_tensor("coop", [BC, N], F32, kind="ExternalOutput").ap()

    with tile.TileContext(nc) as tc:
        _body(tc, zero_bias, obsA, W1, W2, Wh, Wfin, b1, b2, bh, bfin,
              out_cov, out_trk, out_coop)
    nc.compile()
    return nc


def _body(tc, zero_bias, obsA, W1, W2, Wh, Wfin, b1, b2, bh, bfin,
          out_cov, out_trk, out_coop):
    nc = tc.nc

    with (
        tc.tile_pool(name="weights", bufs=1) as wpool,
        tc.tile_pool(name="obs", bufs=1) as obspool,
        tc.tile_pool(name="acts", bufs=3) as actpool,
        tc.tile_pool(name="hpool", bufs=3) as hpool,
        tc.tile_pool(name="gpool", bufs=3) as gpool,
        tc.tile_pool(name="small", bufs=2) as smallpool,
        tc.tile_pool(name="psum", bufs=4, space="PSUM") as psum,
        tc.tile_pool(name="psum_fin", bufs=2, space="PSUM") as psum_f,
        tc.tile_pool(name="psum_tr", bufs=2, space="PSUM") as psum_t,
    ):
        # ---- resident weights / biases ----------------------------------
        obsA_r = obsA.rearrange("p (t k j) -> p t k j", t=NT, k=KO)
        w1_sb = wpool.tile([P, KO, H], F8)
        w2_sb = wpool.tile([P, HO, H], F8)
        wh_sb = wpool.tile([P, HO, 2 * H], F8)

        # SP queue: W1 + obs tile 0 (first matmul's critical path), then the
        # remaining obs tiles prefetched as one 6KB-per-partition DMA.
        x_all = obspool.tile([P, NT, KO, NTILE], F8)
        xs = {t: x_all[:, t] for t in range(NT)}
        nc.sync.dma_start(out=w1_sb, in_=W1.rearrange("p (c h) -> p c h", c=KO))
        nc.sync.dma_start(out=xs[0], in_=obsA_r[:, 0, :, :])
        b1_sb = wpool.tile([P, HO], F32)
        nc.sync.dma_start(out=b1_sb, in_=b1)
        nc.sync.dma_start(out=x_all[:, 1:NT], in_=obsA_r[:, 1:NT, :, :])
        # Activation queue, in consumption order: W2, Wh, finals
        nc.scalar.dma_start(out=w2_sb, in_=W2.rearrange("p (c h) -> p c h", c=HO))
        b2_sb = wpool.tile([P, HO], F32)
        nc.scalar.dma_start(out=b2_sb, in_=b2)
        nc.scalar.dma_start(out=wh_sb, in_=Wh.rearrange("p (c h) -> p c h", c=HO))
        bh_sb = wpool.tile([P, AO], F32)
        nc.scalar.dma_start(out=bh_sb, in_=bh)
        wfin_sb = wpool.tile([P, AO, FINW], F8)
        nc.scalar.dma_start(out=wfin_sb,
                            in_=Wfin.rearrange("p (c m) -> p c m", c=AO))
        bfin_sb = wpool.tile([3, 1], F32)
        nc.scalar.dma_start(out=bfin_sb, in_=bfin.rearrange("(m o) -> m o", o=1))
        ident = wpool.tile([P, P], F32)
        make_identity(nc, ident)

        def act_relu(out, ps, bias_sb, ds, use_dve):
            """out_fp8 = S*relu(z+b): scalar path relu(ps*ds + S*b),
            DVE path (zero bias only) max(ps*ds, 0)."""
            if use_dve and zero_bias:
                nc.vector.tensor_scalar(out, ps, ds, 0.0, MULT, MAX)
            else:
                nc.scalar.activation(out, ps, RELU, bias=bias_sb, scale=ds)

        g1s = {}
        gs = {}

        def layer(x, w_sb, b_sb, ds, out_tag, kchunks):
            """one fused GEMM layer: out = S*relu(w.T @ x + b) in fp8"""
            out = (actpool if out_tag == "g1" else gpool).tile(
                [P, HO, NTILE], F8, tag=out_tag)
            for m in range(HO):
                ps = psum.tile([P, NTILE], F32, tag="mm")
                for h in range(2):
                    cols = slice(h * HALF, (h + 1) * HALF)
                    for k in range(kchunks // 2):
                        nc.tensor.matmul(
                            ps[:, cols],
                            w_sb[:, 2 * k:2 * k + 2, m * P:(m + 1) * P],
                            x[:, 2 * k:2 * k + 2, cols],
                            start=(k == 0), stop=(k == kchunks // 2 - 1),
                            perf_mode=DR)
                act_relu(out[:, m, :], ps, b_sb[:, m:m + 1], ds, m % 2 == 1)
            return out

        def l1(t):
            g1s[t] = layer(xs.pop(t), w1_sb, b1_sb, DS1, "g1", KO)

        def l2(t):
            gs[t] = layer(g1s.pop(t), w2_sb, b2_sb, DS2, "g", HO)

        def heads(t):
            bs = t * NTILE
            g = gs.pop(t)
            # head hiddens h = S3*relu(Wh.T @ g + bh), produced in chunk
            # pairs; each pair feeds one DoubleRow final matmul, emitted one
            # pair LATE so the pair's relu/quant (on scalar/DVE) hides behind
            # the next pair's 8 matmuls instead of stalling the PE.  The two
            # batch halves' fin accumulation groups are interleaved in time,
            # so each needs its own full PSUM bank: a `start` marks the whole
            # 2KB zero region pending and would wipe the other half's partial
            # accumulation (and the ISA requires dst partition base 0).
            ps3 = [psum_f.tile([FINW, NTILE], F32, tag="fin", name=f"fin{t}_{h}")
                   for h in range(2)]
            h2s = {}

            def fin(j):
                for h in range(2):
                    cols = slice(h * HALF, (h + 1) * HALF)
                    nc.tensor.matmul(
                        ps3[h][:, cols],
                        wfin_sb[:, 2 * j:2 * j + 2, :],
                        h2s[j][:, :, cols],
                        start=(j == 0), stop=(j == AO // 2 - 1),
                        perf_mode=DR)

            for m in range(AO):
                j = m // 2
                if m % 2 == 0:
                    h2s[j] = hpool.tile([P, 2, NTILE], F8, tag="h",
                                        name=f"h{t}_{j}")
                ps = psum.tile([P, NTILE], F32, tag="mm")
                for h in range(2):
                    cols = slice(h * HALF, (h + 1) * HALF)
                    for k in range(HO // 2):
                        nc.tensor.matmul(
                            ps[:, cols],
                            wh_sb[:, 2 * k:2 * k + 2, m * P:(m + 1) * P],
                            g[:, 2 * k:2 * k + 2, cols],
                            start=(k == 0), stop=(k == HO // 2 - 1),
                            perf_mode=DR)
                act_relu(h2s[j][:, m % 2, :], ps, bh_sb[:, m:m + 1], DS3,
                         m % 2 == 1)
                if m % 2 == 1 and j >= 1:
                    fin(j - 1)
                    h2s.pop(j - 1)
            fin(AO // 2 - 1)
            sig = smallpool.tile([3, NTILE], F32, tag="sig")
            for h in range(2):
                cols = slice(h * HALF, (h + 1) * HALF)
                nc.scalar.activation(sig[:, cols], ps3[h][0:3, cols],
                                     SIGMOID, bias=bfin_sb[0:3, 0:1],
                                     scale=DSF)
            # transpose to batch-major, broadcast to 9, store the whole tile
            # with one DMA per output tensor (dma_start issue on the SP
            # sequencer costs ~600ns, so 3 instead of 12 per tile)
            o27 = smallpool.tile([P, NTILE // P, 3, N], F32, tag="o27")
            for c in range(NTILE // P):
                pst = psum_t.tile([P, 3], F32, tag="tr")
                nc.tensor.transpose(pst, sig[:, c * P:(c + 1) * P],
                                    ident[0:3, 0:3])
                for i in range(3):
                    nc.vector.tensor_copy(
                        out=o27[:, c, i, :],
                        in_=pst[:, i:i + 1].broadcast_to([P, N]))
            for i, out_t in enumerate((out_cov, out_trk, out_coop)):
                dst = out_t[bs:bs + NTILE, :].rearrange("(c p) n -> p c n", p=P)
                nc.sync.dma_start(out=dst, in_=o27[:, :, i, :])

        # Software-pipelined phase order: every layer phase is separated from
        # its producer phase by at least one unrelated phase, so the
        # producer's last relu/quant (scalar/DVE) lands well before the
        # consumer's PE matmuls need it — no layer-boundary PE stalls.
        l1(0)
        l1(1)
        l2(0)
        l1(2)
        l2(1)
        heads(0)
        l1(3)
        l2(2)
        heads(1)
        l2(3)
        heads(2)
        heads(3)


_NC_CACHE = {}


def _get_nc(zero_bias: bool = True) -> bass.Bass:
    if zero_bias not in _NC_CACHE:
        _NC_CACHE[zero_bias] = build_nc(zero_bias)
    return _NC_CACHE[zero_bias]


def prep_inputs(obs, W1, b1, W2, b2, Wc1, bc1, Wc2, bc2,
                Wt1, bt1, Wt2, bt2, Wk1, bk1, Wk2, bk2, **_unused):
    """Host-side prep: fold/concat weights, scale + quantise to fp8,
    transpose obs, build shards."""
    f = np.float32

    def q8(a):
        return np.ascontiguousarray(
            np.clip(np.asarray(a, f), -240.0, 240.0).astype(NP_F8))

    Wk1f = np.asarray(Wk1[:H], f) + np.asarray(Wk1[H:], f)     # [H, H]
    Wh = np.concatenate([np.asarray(Wc1, f), np.asarray(Wt1, f), Wk1f],
                        axis=1)                                # [H, 2H]
    Wfin = np.zeros((2 * H, FINW), f)
    Wfin[0:H // 2, 0] = np.asarray(Wc2, f)[:, 0]
    Wfin[H // 2:H, 1] = np.asarray(Wt2, f)[:, 0]
    Wfin[H:2 * H, 2] = np.asarray(Wk2, f)[:, 0]
    bhc = np.concatenate([np.asarray(bc1, f), np.asarray(bt1, f),
                          np.asarray(bk1, f)])                 # [2H]
    bfin = np.array([np.asarray(bc2, f)[0], np.asarray(bt2, f)[0],
                     np.asarray(bk2, f)[0]], f)

    # biases folded at each layer's activation quantisation scale,
    # pre-arranged to [partition, chunk]
    def barr(bq, chunks):
        return np.ascontiguousarray(
            bq.reshape(chunks, P).T.astype(f))           # [P, chunks]

    b1q = (S1 * np.asarray(b1, f)).astype(f)
    b2q = (S2 * np.asarray(b2, f)).astype(f)
    bhq = (S3 * bhc).astype(f)
    zero_bias = not (b1q.any() or b2q.any() or bhq.any())

    # weights pre-arranged to [partition, k_chunk * out]: A[p, c*out+o]
    # = Wq[c*128+p, o], so each partition's SBUF row is one contiguous run
    def warr(wq, chunks, out):
        return np.ascontiguousarray(
            wq.reshape(chunks, P, out).transpose(1, 0, 2).reshape(P, -1))

    shared = dict(
        W1A=warr(q8(np.asarray(W1, f) * WS), KO, H),
        W2A=warr(q8(np.asarray(W2, f) * WS), HO, H),
        WhA=warr(q8(Wh * WS), HO, 2 * H),
        WfinA=warr(q8(Wfin * WS), AO, FINW),
        b1A=barr(b1q, HO), b2A=barr(b2q, HO), bhA=barr(bhq, AO),
        bfin=bfin,
    )
    # obs pre-arranged per core to [p, t*KO*NTILE + k*NTILE + j]
    # = obs_q[core*BC + t*NTILE + j, k*128+p]
    obs_q = np.clip(np.asarray(obs, f), -240.0, 240.0).astype(NP_F8)
    in_maps = []
    for c in range(NCORES):
        m = dict(shared)
        blk = obs_q[c * BC:(c + 1) * BC]                 # [BC, OBS]
        m["obsA"] = np.ascontiguousarray(
            blk.reshape(NT, NTILE, KO, P).transpose(3, 0, 2, 1).reshape(P, -1))
        in_maps.append(m)
    return in_maps, zero_bias


def kernel(**inputs):
    in_maps, zero_bias = prep_inputs(**inputs)
    nc = _get_nc(zero_bias)
    res = run_bass_kernel_spmd(nc, in_maps, list(range(NCORES))).results
    cov = np.concatenate([res[c]["cov"] for c in range(NCORES)], axis=0)
    trk = np.concatenate([res[c]["trk"] for c in range(NCORES)], axis=0)
    coop = np.concatenate([res[c]["coop"] for c in range(NCORES)], axis=0)
    return (cov, trk, coop)
